# revision 22
# baseline (speedup 1.0000x reference)
"""GQA (grouped-query attention) Trainium2 kernel, 8 NeuronCores.

Sharding v2: core c = (batch b = c//4, kv-group g = c%4). Each core computes
the 4 query heads of kv group g for batch b only (2048 tokens), with wo
row-sharded over those heads; the 4 cores of each batch return bf16 partial
[S, HID] outputs that the host sums in f32. vs. head-only sharding this
halves per-core x traffic, removes duplicated K/V projection work, and halves
the partial-output write.

Data path is bf16 (inputs converted host-side; f32 accumulation in PSUM):
validated at ~6e-3 rel err vs the f32 reference (gate 2e-2).

Per-core device kernel, interleaved by 512-token block J:
  proj(J):  J=0 runs a single 6-bank pass (q0..q3, k interleaved per ko, then
            V) so PE consumption rides the x DMA-chunk arrival; J>0 runs
            three 2-bank passes (q0,q1), (q2,q3), (k, V) from SBUF-resident
            x^T. RoPE is split into a PSUM-releasing staging copy + math
            (DVE) into bf16 q/k; V is produced directly in natural [s,d]
            layout by using x^T slices as the matmul stationary.
  attn(J):  per head, software-pipelined with a one-block skew:
            scores^T = K_blk q (PE) -> exp bf16 (ACT, key_weight*scale folded
            into the activation scale) -> causal mask on diagonal blocks
            (Pool affine_select) -> attn^T accum + sum-of-exp via ones-matmul
            (PE) one block behind. Normalization of head h is deferred into
            head h+1's first block (fills the exp-latency bubble): 1/sum
            broadcast by a K=1 PE outer product, applied on DVE into bf16.
  wo(J):    deferred by one block (so wo weights can load last and its PE
            work overlaps attention): per 128-row block, 4x4 (head x
            embed-chunk) PE accumulation, copies (ACT/DVE alternating) to a
            [128, 2048] bf16 staging tile, one DMA per row block.

PSUM budget: pj0+pj1 (proj) + 3 scores/bcast/wo + 2 av + 1 sums = 8 banks.
DMA issue order follows first-use time; TimelineSim predicts ~282us/core
(PE busy ~231us) vs ~403us for the f32r head-sharded baseline.
"""
import numpy as np
import ml_dtypes

B, S, HID = 2, 2048, 2048
NH, NKV, D = 16, 4, 128
NCORES = 8
HPC = 4                       # q heads per core (one kv group)
NKO = HID // 128              # 16 contraction chunks
NJ = S // 512                 # 4 q/seq blocks per core
ROPE_BASE = 10000.0
SCALE = float(D) ** -0.5
BF16 = ml_dtypes.bfloat16

_cache = {}


def _consts():
    half = D // 2
    pos = np.arange(S, dtype=np.float32)
    inv_freq = (1.0 / (ROPE_BASE ** (np.arange(half, dtype=np.float32) / np.float32(half)))).astype(np.float32)
    ang = pos[:, None] * inv_freq[None, :]              # [S, 64]
    cos = np.cos(ang).astype(np.float32).T              # [64, S]
    sin = np.sin(ang).astype(np.float32).T
    cos_full = np.concatenate([cos, cos], 0).astype(BF16)        # [128, S]
    sinsw = np.concatenate([sin, -sin], 0).astype(BF16)          # [128, S] (halves pre-swapped)
    ones_col = np.ones((128, 1), BF16)
    ones_row = np.ones((1, 128), np.float32)
    return cos_full, sinsw, ones_col, ones_row


def _build():
    import concourse.mybir as mybir
    from concourse import bacc
    from concourse.tile import TileContext

    f32 = mybir.dt.float32
    f32r = mybir.dt.float32r
    bf16 = mybir.dt.bfloat16
    MUL = mybir.AluOpType.mult
    ADD = mybir.AluOpType.add
    EXP = mybir.ActivationFunctionType.Exp
    CPY = mybir.ActivationFunctionType.Copy

    cos_np, sinsw_np, onescol_np, onesrow_np = _consts()

    nc = bacc.Bacc("TRN2", target_bir_lowering=False, debug=False)

    xT = nc.dram_tensor("xT", [HID, S], bf16, kind="ExternalInput")
    wqT = nc.dram_tensor("wqT", [HID, HPC * D], bf16, kind="ExternalInput")
    wkT = nc.dram_tensor("wkT", [HID, D], bf16, kind="ExternalInput")
    wvT = nc.dram_tensor("wvT", [HID, D], bf16, kind="ExternalInput")
    woT = nc.dram_tensor("woT", [HPC * D, HID], bf16, kind="ExternalInput")
    kw = nc.dram_tensor("kw", [HPC], f32, kind="ExternalInput")
    out = nc.dram_tensor("out", [S, HID], bf16, kind="ExternalOutput")

    cos_d = nc.inline_tensor(cos_np, name="cos_t")
    sinsw_d = nc.inline_tensor(sinsw_np, name="sinsw_t")
    onescol_d = nc.inline_tensor(onescol_np, name="onescol_t")
    onesrow_d = nc.inline_tensor(onesrow_np, name="onesrow_t")

    with TileContext(nc) as tc:
        with tc.tile_pool(name="persist", bufs=1) as pp, \
             tc.tile_pool(name="pj", bufs=1, space="PSUM") as pj, \
             tc.tile_pool(name="scps", bufs=3, space="PSUM") as scps, \
             tc.tile_pool(name="avps", bufs=2, space="PSUM") as avps, \
             tc.tile_pool(name="sups", bufs=1, space="PSUM") as sups, \
             tc.tile_pool(name="rt", bufs=3) as rt, \
             tc.tile_pool(name="rcs", bufs=5) as rcs, \
             tc.tile_pool(name="ep", bufs=6) as ep, \
             tc.tile_pool(name="bp", bufs=3) as bp, \
             tc.tile_pool(name="ap", bufs=1) as ap, \
             tc.tile_pool(name="smp", bufs=3) as smp, \
             tc.tile_pool(name="fsb", bufs=3) as fsb:
            # persistent SBUF tensors
            x_sb = pp.tile([128, NKO, S], bf16, tag="x")
            wq_sb = pp.tile([128, NKO, HPC * D], bf16, tag="wq")
            wk_sb = pp.tile([128, NKO, D], bf16, tag="wk")
            wv_sb = pp.tile([128, NKO, D], bf16, tag="wv")
            wo_sb = pp.tile([128, HPC, HID], bf16, tag="wo")
            q_sb = [pp.tile([128, S], bf16, tag=f"q{h}", name=f"q_sb{h}") for h in range(HPC)]
            k_sb = pp.tile([128, S], bf16, tag="k")
            v_sb = pp.tile([128, S // 128, D], bf16, tag="v")
            cos_sb = pp.tile([128, S], bf16, tag="cos")
            sinsw_sb = pp.tile([128, S], bf16, tag="sinsw")
            onescol_sb = pp.tile([128, 1], bf16, tag="onescol")
            onesrow_sb = pp.tile([1, 128], f32r, tag="onesrow")
            kwsc_sb = pp.tile([128, HPC], f32, tag="kwsc")
            kwraw_sb = pp.tile([128, HPC], f32, tag="kwraw")

            xr = xT[:].rearrange("(ko p) s -> p ko s", p=128)
            wqr = wqT[:].rearrange("(ko p) d -> p ko d", p=128)
            # DMA issue order follows first-use time: wq halves and x gate the
            # first projection pass, cos/sin gate its rope, wk/wv gate pass C,
            # wo is not needed until wo(0) fires during the J=1 section.
            nc.sync.dma_start(wq_sb[:, :, 0:2 * D], wqr[:, :, 0:2 * D])
            nc.sync.dma_start(wk_sb[:], wkT[:].rearrange("(ko p) d -> p ko d", p=128))
            nc.sync.dma_start(wv_sb[:], wvT[:].rearrange("(ko p) d -> p ko d", p=128))
            for cch in range(8):
                nc.sync.dma_start(x_sb[:, 2 * cch:2 * cch + 2, :], xr[:, 2 * cch:2 * cch + 2, :])
                if cch == 0:
                    nc.sync.dma_start(wq_sb[:, :, 2 * D:4 * D], wqr[:, :, 2 * D:4 * D])
                elif cch == 1:
                    nc.sync.dma_start(cos_sb[:], cos_d[:])
                    nc.sync.dma_start(sinsw_sb[:], sinsw_d[:])
                elif cch == 3:
                    nc.sync.dma_start(onescol_sb[:], onescol_d[:])
                    nc.sync.dma_start(onesrow_sb[:], onesrow_d[:].bitcast(f32r))
                    nc.sync.dma_start(kwraw_sb[:], kw[None, :].to_broadcast((128, HPC)))
                    nc.vector.tensor_scalar_mul(kwsc_sb[:], kwraw_sb[:], SCALE)
            nc.sync.dma_start(wo_sb[:], woT[:].rearrange("(h p) e -> p h e", p=128))

            def rope_cs(src_ps):
                # stage PSUM->SBUF so the projection bank frees after one DVE
                # op; the rope math reads the staged copy later
                cs = rcs.tile([128, 512], f32, tag="cs")
                nc.vector.tensor_copy(cs[:], src_ps[:])
                return cs

            def rope_math(dst, cs, s0):
                t0 = rt.tile([128, 512], f32, tag="t0")
                t1 = rt.tile([128, 512], f32, tag="t1")
                nc.vector.tensor_tensor(t0[:], cs[:], cos_sb[:, s0:s0 + 512], MUL)
                nc.vector.tensor_tensor(t1[0:64, :], cs[64:128, :], sinsw_sb[64:128, s0:s0 + 512], MUL)
                nc.vector.tensor_tensor(t1[64:128, :], cs[0:64, :], sinsw_sb[0:64, s0:s0 + 512], MUL)
                nc.vector.tensor_tensor(dst, t0[:], t1[:], ADD)

            at_sb = [[ap.tile([128, 512], bf16, tag=f"at{p}{h}", name=f"at_sb{p}{h}")
                      for h in range(HPC)] for p in range(2)]

            def norm_head(J, h, avp, sup):
                """softmax normalization of head h of block J: 1/sum broadcast
                via K=1 PE outer product, applied on DVE into bf16 at tile."""
                sums = smp.tile([1, 512], f32, tag="sums")
                nc.vector.tensor_copy(sums[:], sup[:])
                recip = smp.tile([1, 512], f32r, tag="recip")
                with nc.allow_low_precision(reason="f32r rounding of softmax denom, ~1e-4 rel"):
                    nc.vector.reciprocal(recip[:], sums[:])
                bcp = scps.tile([128, 512], f32, tag="sc", name="bcp")
                nc.tensor.matmul(bcp[:], onesrow_sb[:], recip[:], start=True, stop=True)
                bcs = bp.tile([128, 512], f32, tag="bcs")
                nc.scalar.activation(bcs[:], bcp[:], CPY)
                nc.vector.tensor_tensor(at_sb[J % 2][h][:], avp[:], bcs[:], MUL)

            def wo_block(J):
                """output projection for block J (deferred one block so its
                weights load late and its PE work overlaps attention's ACT)."""
                s0 = J * 512
                for i in range(4):
                    ost = fsb.tile([128, HID], bf16, tag="fo")
                    for e in range(4):
                        fp = scps.tile([128, 512], f32, tag="sc", name="fp")
                        for h in range(HPC):
                            nc.tensor.matmul(fp[:], at_sb[J % 2][h][:, i * 128:(i + 1) * 128],
                                             wo_sb[:, h, e * 512:(e + 1) * 512],
                                             start=(h == 0), stop=(h == HPC - 1))
                        if e % 2 == 0:
                            nc.vector.tensor_copy(ost[:, e * 512:(e + 1) * 512], fp[:])
                        else:
                            nc.scalar.activation(ost[:, e * 512:(e + 1) * 512], fp[:], CPY)
                    nc.sync.dma_start(out[s0 + i * 128:s0 + (i + 1) * 128, :], ost[:])

            pend = None   # (J, h, avp, sup) of the head awaiting normalization
            for J in range(NJ):
                s0 = J * 512
                # ---- projections for block J ----
                if J == 0:
                    # Single interleaved pass over ko for q0..q3 + k, borrowing
                    # the (still idle) attention "sc" banks: consumption rides
                    # the x-chunk DMA arrival instead of replaying x 3 times.
                    qps = [pj.tile([128, 512], f32, tag="pj0", name="qps0"),
                           pj.tile([128, 512], f32, tag="pj1", name="qps1"),
                           scps.tile([128, 512], f32, tag="sc", name="qps2"),
                           scps.tile([128, 512], f32, tag="sc", name="qps3")]
                    kps0 = scps.tile([128, 512], f32, tag="sc", name="kps0")
                    for ko in range(NKO):
                        st, sp = (ko == 0), (ko == NKO - 1)
                        xt = x_sb[:, ko, s0:s0 + 512]
                        for h in range(HPC):
                            nc.tensor.matmul(qps[h][:], wq_sb[:, ko, h * D:(h + 1) * D], xt, start=st, stop=sp)
                        nc.tensor.matmul(kps0[:], wk_sb[:, ko, :], xt, start=st, stop=sp)
                    vps0 = avps.tile([128, 512], f32, tag="av", name="vps0")
                    for sblk in range(4):
                        for ko in range(NKO):
                            st, sp = (ko == 0), (ko == NKO - 1)
                            nc.tensor.matmul(vps0[:, sblk * 128:(sblk + 1) * 128],
                                             x_sb[:, ko, s0 + sblk * 128:s0 + (sblk + 1) * 128],
                                             wv_sb[:, ko, :], start=st, stop=sp)
                    csq = [rope_cs(qps[h]) for h in range(HPC)]
                    csk = rope_cs(kps0)
                    rope_math(q_sb[0][:, s0:s0 + 512], csq[0], s0)
                    rope_math(k_sb[:, s0:s0 + 512], csk, s0)
                    for h in range(1, HPC):
                        rope_math(q_sb[h][:, s0:s0 + 512], csq[h], s0)
                    nc.scalar.activation(v_sb[:, J * 4:(J + 1) * 4, :], vps0[:], CPY)
                else:
                    # pass A: q0, q1;  pass B: q2, q3;  pass C: k then v-natural.
                    # Rope is split so the cheap staging copies (which free the
                    # pj banks for the next pass) jump the DVE queue ahead of
                    # the rope math.
                    def qpass(h0, h1):
                        t_a = pj.tile([128, 512], f32, tag="pj0", name="pja")
                        t_b = pj.tile([128, 512], f32, tag="pj1", name="pjb")
                        for ko in range(NKO):
                            st, sp = (ko == 0), (ko == NKO - 1)
                            xt = x_sb[:, ko, s0:s0 + 512]
                            nc.tensor.matmul(t_a[:], wq_sb[:, ko, h0 * D:(h0 + 1) * D], xt, start=st, stop=sp)
                            nc.tensor.matmul(t_b[:], wq_sb[:, ko, h1 * D:(h1 + 1) * D], xt, start=st, stop=sp)
                        return rope_cs(t_a), rope_cs(t_b)
                    cs0, cs1 = qpass(0, 1)
                    cs2, cs3 = qpass(2, 3)
                    rope_math(q_sb[0][:, s0:s0 + 512], cs0, s0)
                    rope_math(q_sb[1][:, s0:s0 + 512], cs1, s0)
                    kps = pj.tile([128, 512], f32, tag="pj0", name="kps")
                    for ko in range(NKO):
                        st, sp = (ko == 0), (ko == NKO - 1)
                        nc.tensor.matmul(kps[:], wk_sb[:, ko, :], x_sb[:, ko, s0:s0 + 512], start=st, stop=sp)
                    csk = rope_cs(kps)
                    rope_math(k_sb[:, s0:s0 + 512], csk, s0)
                    vps = pj.tile([128, 512], f32, tag="pj1", name="vps")
                    for sblk in range(4):
                        for ko in range(NKO):
                            st, sp = (ko == 0), (ko == NKO - 1)
                            nc.tensor.matmul(vps[:, sblk * 128:(sblk + 1) * 128],
                                             x_sb[:, ko, s0 + sblk * 128:s0 + (sblk + 1) * 128],
                                             wv_sb[:, ko, :], start=st, stop=sp)
                    rope_math(q_sb[2][:, s0:s0 + 512], cs2, s0)
                    rope_math(q_sb[3][:, s0:s0 + 512], cs3, s0)
                    nc.scalar.activation(v_sb[:, J * 4:(J + 1) * 4, :], vps[:], CPY)

                # ---- attention for block J ----
                nkb = 4 * J + 4
                for h in range(HPC):
                    avp = avps.tile([128, 512], f32, tag="av", name="avp")
                    sup = None
                    exq = []   # (jj, ex, off, n) awaiting their AV/sum matmuls

                    def flush_one():
                        nonlocal sup
                        jj0, ex0, off0, n0 = exq.pop(0)
                        st0, sp0 = (jj0 == 0), (jj0 == nkb - 1)
                        nc.tensor.matmul(avp[:, off0:512], v_sb[:, jj0, :], ex0[:, 0:n0], start=st0, stop=sp0)
                        if jj0 == 0:
                            sup = sups.tile([1, 512], f32, tag="su", name="sup")
                        nc.tensor.matmul(sup[:, off0:512], onescol_sb[:], ex0[:, 0:n0], start=st0, stop=sp0)

                    for jj in range(nkb):
                        p = jj - 4 * J
                        off = max(p, 0) * 128
                        n = 512 - off
                        scp = scps.tile([128, 512], f32, tag="sc", name="scp")
                        nc.tensor.matmul(scp[:, 0:n], k_sb[:, jj * 128:(jj + 1) * 128],
                                         q_sb[h][:, s0 + off:s0 + 512], start=True, stop=True)
                        if jj == 0 and pend is not None:
                            # normalize the previous head here: its PE op fills
                            # the exp-latency bubble of this head's first block
                            norm_head(*pend)
                            pend = None
                        ex = ep.tile([128, 512], bf16, tag="ex")
                        nc.scalar.activation(ex[:, 0:n], scp[:, 0:n], EXP,
                                             scale=kwsc_sb[:, h:h + 1])
                        if p >= 0:
                            # keep where col - row >= 0 (causal, off = p*128 aligns it)
                            nc.gpsimd.affine_select(
                                ex[:, 0:n], ex[:, 0:n], pattern=[[1, n]],
                                compare_op=mybir.AluOpType.is_ge, fill=0.0,
                                base=0, channel_multiplier=-1)
                        exq.append((jj, ex, off, n))
                        if jj >= 1:
                            # one-block skew: AV/sum of block jj-1 run while
                            # exp(jj) is still in flight on ACT
                            flush_one()
                    while exq:
                        flush_one()
                    pend = (J, h, avp, sup)

                # ---- deferred output projection of the previous block ----
                if J > 0:
                    wo_block(J - 1)
                norm_head(*pend)
                pend = None
            wo_block(NJ - 1)

    nc.compile()
    return nc


def _get_exec():
    """Build the Bass module once and wrap it in a cached jitted shard_map
    executable (mirrors concourse.bass2jax.run_bass_via_pjrt, minus donation so
    repeated calls can reuse device-resident buffers)."""
    if "exec" in _cache:
        return _cache["exec"]
    import jax
    import concourse.mybir as mybir
    from jax.experimental.shard_map import shard_map
    from jax.sharding import Mesh, PartitionSpec
    from concourse import bass2jax

    nc = _build()
    bass2jax.install_neuronx_cc_hook()

    partition_name = nc.partition_id_tensor.name if nc.partition_id_tensor else None
    in_names, out_names, out_avals = [], [], []
    for alloc in nc.m.functions[0].allocations:
        if not isinstance(alloc, mybir.__dict__["MemoryLocationSet"]):
            continue
        name = alloc.memorylocations[0].name
        if alloc.kind == "ExternalInput":
            if name != partition_name:
                in_names.append(name)
        elif alloc.kind == "ExternalOutput":
            out_names.append(name)
            out_avals.append(jax.core.ShapedArray(tuple(alloc.tensor_shape),
                                                  mybir.dt.np(alloc.dtype)))
    n_params = len(in_names)
    in_names = in_names + out_names  # zero-buffer operands, per bass2jax contract
    if partition_name is not None:
        in_names.append(partition_name)

    def _body(*args):
        operands = list(args)
        if partition_name is not None:
            operands.append(bass2jax.partition_id_tensor())
        outs = bass2jax._bass_exec_p.bind(
            *operands,
            out_avals=tuple(out_avals),
            in_names=tuple(in_names),
            out_names=tuple(out_names),
            lowering_input_output_aliases=(),
            sim_require_finite=True,
            sim_require_nnan=True,
            nc=nc,
        )
        return tuple(outs)

    devices = jax.devices()[:NCORES]
    mesh = Mesh(np.asarray(devices), ("core",))
    spec = PartitionSpec("core")
    sharded = jax.jit(
        shard_map(_body, mesh=mesh,
                  in_specs=(spec,) * (n_params + len(out_names)),
                  out_specs=(spec,) * len(out_names),
                  check_rep=False),
        keep_unused=True,
    )
    _cache["exec"] = {
        "sharded": sharded, "in_names": in_names, "out_names": out_names,
        "out_avals": out_avals, "n_params": n_params, "mesh": mesh, "spec": spec,
    }
    return _cache["exec"]


def _prep_in_maps(x, wq, wk, wv, wo, key_weights):
    x = np.asarray(x, dtype=np.float32)
    wq = np.asarray(wq, dtype=np.float32)
    wk = np.asarray(wk, dtype=np.float32)
    wv = np.asarray(wv, dtype=np.float32)
    wo = np.asarray(wo, dtype=np.float32)
    key_weights = np.asarray(key_weights, dtype=np.float32)

    wqT = np.ascontiguousarray(wq.T).astype(BF16)            # [HID, NH*D]
    wkT = np.ascontiguousarray(wk.T).astype(BF16)            # [HID, NKV*D]
    wvT = np.ascontiguousarray(wv.T).astype(BF16)
    woT = np.ascontiguousarray(wo.T).astype(BF16)            # [NH*D, HID]
    xTb = [np.ascontiguousarray(x[b].T).astype(BF16) for b in range(B)]

    in_maps = []
    for c in range(NCORES):
        b, g = c // NKV, c % NKV
        in_maps.append({
            "xT": xTb[b],
            "wqT": np.ascontiguousarray(wqT[:, g * HPC * D:(g + 1) * HPC * D]),
            "wkT": np.ascontiguousarray(wkT[:, g * D:(g + 1) * D]),
            "wvT": np.ascontiguousarray(wvT[:, g * D:(g + 1) * D]),
            "woT": np.ascontiguousarray(woT[g * HPC * D:(g + 1) * HPC * D, :]),
            "kw": np.ascontiguousarray(key_weights[g * HPC:(g + 1) * HPC]),
        })
    return in_maps


def _concat_args(ex, in_maps):
    concat_in = [
        np.concatenate([np.asarray(in_maps[c][name]) for c in range(NCORES)], axis=0)
        for name in ex["in_names"][:ex["n_params"]]
    ]
    zeros = [
        np.zeros((NCORES * av.shape[0], *av.shape[1:]), av.dtype)
        for av in ex["out_avals"]
    ]
    return concat_in + zeros


def kernel(x, wq, wk, wv, wo, key_weights):
    ex = _get_exec()
    in_maps = _prep_in_maps(x, wq, wk, wv, wo, key_weights)
    args = _concat_args(ex, in_maps)
    out_arrs = ex["sharded"](*args)
    parts = np.asarray(out_arrs[0]).astype(np.float32).reshape(B, NKV, S, HID)
    return parts.sum(axis=1)


# revision 26
# speedup vs baseline: 1.0519x; 1.0519x over previous
"""GQA (grouped-query attention) Trainium2 kernel, 8 NeuronCores.

Sharding v2: core c = (batch b = c//4, kv-group g = c%4). Each core computes
the 4 query heads of kv group g for batch b only (2048 tokens), with wo
row-sharded over those heads; the 4 cores of each batch return bf16 partial
[S, HID] outputs that the host sums in f32. vs. head-only sharding this
halves per-core x traffic, removes duplicated K/V projection work, and halves
the partial-output write.

Data path is bf16 (inputs converted host-side; f32 accumulation in PSUM):
validated at ~6e-3 rel err vs the f32 reference (gate 2e-2).

Per-core device kernel, interleaved by 512-token block J:
  proj(J):  J=0 runs a single 6-bank pass (q0..q3, k interleaved per ko, then
            V) so PE consumption rides the x DMA-chunk arrival; J>0 runs
            three 2-bank passes (q0,q1), (q2,q3), (k, V) from SBUF-resident
            x^T. RoPE is split into a PSUM-releasing staging copy + math
            (DVE) into bf16 q/k; V is produced directly in natural [s,d]
            layout by using x^T slices as the matmul stationary.
  attn(J):  per head, software-pipelined with a one-block skew:
            scores^T = K_blk q (PE) -> exp bf16 (ACT, key_weight*scale folded
            into the activation scale) -> causal mask on diagonal blocks
            (Pool affine_select) -> attn^T accum + sum-of-exp via ones-matmul
            (PE) one block behind. Normalization of head h is deferred into
            head h+1's first block (fills the exp-latency bubble): 1/sum
            broadcast by a K=1 PE outer product, applied on DVE into bf16.
  wo(J):    deferred by one block (so wo weights can load last and its PE
            work overlaps attention): per 128-row block, 4x4 (head x
            embed-chunk) PE accumulation, copies (ACT/DVE alternating) to a
            [128, 2048] bf16 staging tile, one DMA per row block.

PSUM budget: pj0+pj1 (proj) + 3 scores/bcast/wo + 2 av + 1 sums = 8 banks.
DMA issue order follows first-use time; TimelineSim predicts ~282us/core
(PE busy ~231us) vs ~403us for the f32r head-sharded baseline.
"""
import numpy as np
import ml_dtypes

B, S, HID = 2, 2048, 2048
NH, NKV, D = 16, 4, 128
NCORES = 8
HPC = 4                       # q heads per core (one kv group)
NKO = HID // 128              # 16 contraction chunks
NJ = S // 512                 # 4 q/seq blocks per core
ROPE_BASE = 10000.0
SCALE = float(D) ** -0.5
BF16 = ml_dtypes.bfloat16

_cache = {}


def _consts():
    half = D // 2
    pos = np.arange(S, dtype=np.float32)
    inv_freq = (1.0 / (ROPE_BASE ** (np.arange(half, dtype=np.float32) / np.float32(half)))).astype(np.float32)
    ang = pos[:, None] * inv_freq[None, :]              # [S, 64]
    cos = np.cos(ang).astype(np.float32).T              # [64, S]
    sin = np.sin(ang).astype(np.float32).T
    cos_full = np.concatenate([cos, cos], 0).astype(BF16)        # [128, S]
    sinsw = np.concatenate([sin, -sin], 0).astype(BF16)          # [128, S] (halves pre-swapped)
    ones_col = np.ones((128, 1), BF16)
    ones_row = np.ones((1, 128), np.float32)
    return cos_full, sinsw, ones_col, ones_row


def _build():
    import concourse.mybir as mybir
    from concourse import bacc
    from concourse.tile import TileContext

    f32 = mybir.dt.float32
    f32r = mybir.dt.float32r
    bf16 = mybir.dt.bfloat16
    MUL = mybir.AluOpType.mult
    ADD = mybir.AluOpType.add
    EXP = mybir.ActivationFunctionType.Exp
    CPY = mybir.ActivationFunctionType.Copy

    cos_np, sinsw_np, onescol_np, onesrow_np = _consts()

    nc = bacc.Bacc("TRN2", target_bir_lowering=False, debug=False)

    xT = nc.dram_tensor("xT", [HID, S], bf16, kind="ExternalInput")
    wqT = nc.dram_tensor("wqT", [HID, HPC * D], bf16, kind="ExternalInput")
    wkT = nc.dram_tensor("wkT", [HID, D], bf16, kind="ExternalInput")
    wvT = nc.dram_tensor("wvT", [HID, D], bf16, kind="ExternalInput")
    woT = nc.dram_tensor("woT", [HPC * D, HID], bf16, kind="ExternalInput")
    kw = nc.dram_tensor("kw", [HPC], f32, kind="ExternalInput")
    out = nc.dram_tensor("out", [S, HID], bf16, kind="ExternalOutput")

    cos_d = nc.inline_tensor(cos_np, name="cos_t")
    sinsw_d = nc.inline_tensor(sinsw_np, name="sinsw_t")
    onescol_d = nc.inline_tensor(onescol_np, name="onescol_t")
    onesrow_d = nc.inline_tensor(onesrow_np, name="onesrow_t")

    with TileContext(nc) as tc:
        with tc.tile_pool(name="persist", bufs=1) as pp, \
             tc.tile_pool(name="pj", bufs=1, space="PSUM") as pj, \
             tc.tile_pool(name="scps", bufs=3, space="PSUM") as scps, \
             tc.tile_pool(name="avps", bufs=2, space="PSUM") as avps, \
             tc.tile_pool(name="sups", bufs=1, space="PSUM") as sups, \
             tc.tile_pool(name="rt", bufs=3) as rt, \
             tc.tile_pool(name="rcs", bufs=5) as rcs, \
             tc.tile_pool(name="ep", bufs=6) as ep, \
             tc.tile_pool(name="bp", bufs=3) as bp, \
             tc.tile_pool(name="ap", bufs=1) as ap, \
             tc.tile_pool(name="smp", bufs=3) as smp, \
             tc.tile_pool(name="fsb", bufs=3) as fsb:
            # persistent SBUF tensors
            x_sb = pp.tile([128, NKO, S], bf16, tag="x")
            wq_sb = pp.tile([128, NKO, HPC * D], bf16, tag="wq")
            wk_sb = pp.tile([128, NKO, D], bf16, tag="wk")
            wv_sb = pp.tile([128, NKO, D], bf16, tag="wv")
            wo_sb = pp.tile([128, HPC, HID], bf16, tag="wo")
            q_sb = [pp.tile([128, S], bf16, tag=f"q{h}", name=f"q_sb{h}") for h in range(HPC)]
            k_sb = pp.tile([128, S], bf16, tag="k")
            v_sb = pp.tile([128, S // 128, D], bf16, tag="v")
            cos_sb = pp.tile([128, S], bf16, tag="cos")
            sinsw_sb = pp.tile([128, S], bf16, tag="sinsw")
            onescol_sb = pp.tile([128, 1], bf16, tag="onescol")
            onesrow_sb = pp.tile([1, 128], f32r, tag="onesrow")
            kwsc_sb = pp.tile([128, HPC], f32, tag="kwsc")
            kwraw_sb = pp.tile([128, HPC], f32, tag="kwraw")

            xr = xT[:].rearrange("(ko p) s -> p ko s", p=128)
            wqr = wqT[:].rearrange("(ko p) d -> p ko d", p=128)
            # DMA issue order follows first-use time: wq halves and x gate the
            # first projection pass, cos/sin gate its rope, wk/wv gate pass C,
            # wo is not needed until wo(0) fires during the J=1 section.
            # first-use-ordered loads, finest at the front: the first
            # projection matmuls need only q0's weight column block, wk, and
            # the first quarter of x slice 0.
            nc.sync.dma_start(wq_sb[:, :, 0:D], wqr[:, :, 0:D])
            nc.sync.dma_start(wk_sb[:], wkT[:].rearrange("(ko p) d -> p ko d", p=128))
            nc.sync.dma_start(x_sb[:, 0:4, 0:512], xr[:, 0:4, 0:512])
            nc.sync.dma_start(wq_sb[:, :, D:2 * D], wqr[:, :, D:2 * D])
            nc.sync.dma_start(x_sb[:, 4:8, 0:512], xr[:, 4:8, 0:512])
            nc.sync.dma_start(wv_sb[:], wvT[:].rearrange("(ko p) d -> p ko d", p=128))
            # x is chunked by sequence slice: block J's entire projection only
            # needs slice J, so J=0 compute starts almost immediately.
            for sj in range(4):
                if sj > 0:
                    nc.sync.dma_start(x_sb[:, 0:8, 512 * sj:512 * (sj + 1)],
                                      xr[:, 0:8, 512 * sj:512 * (sj + 1)])
                nc.sync.dma_start(x_sb[:, 8:16, 512 * sj:512 * (sj + 1)],
                                  xr[:, 8:16, 512 * sj:512 * (sj + 1)])
                if sj == 0:
                    nc.sync.dma_start(wq_sb[:, :, 2 * D:4 * D], wqr[:, :, 2 * D:4 * D])
                    nc.sync.dma_start(cos_sb[:], cos_d[:])
                    nc.sync.dma_start(sinsw_sb[:], sinsw_d[:])
                elif sj == 1:
                    nc.sync.dma_start(onescol_sb[:], onescol_d[:])
                    nc.sync.dma_start(onesrow_sb[:], onesrow_d[:].bitcast(f32r))
                    nc.sync.dma_start(kwraw_sb[:], kw[None, :].to_broadcast((128, HPC)))
                    nc.vector.tensor_scalar_mul(kwsc_sb[:], kwraw_sb[:], SCALE)
            nc.sync.dma_start(wo_sb[:], woT[:].rearrange("(h p) e -> p h e", p=128))

            def rope_cs(src_ps):
                # stage PSUM->SBUF so the projection bank frees after one DVE
                # op; the rope math reads the staged copy later
                cs = rcs.tile([128, 512], f32, tag="cs")
                nc.vector.tensor_copy(cs[:], src_ps[:])
                return cs

            def rope_math(dst, cs, s0):
                t0 = rt.tile([128, 512], f32, tag="t0")
                t1 = rt.tile([128, 512], f32, tag="t1")
                nc.vector.tensor_tensor(t0[:], cs[:], cos_sb[:, s0:s0 + 512], MUL)
                nc.vector.tensor_tensor(t1[0:64, :], cs[64:128, :], sinsw_sb[64:128, s0:s0 + 512], MUL)
                nc.vector.tensor_tensor(t1[64:128, :], cs[0:64, :], sinsw_sb[0:64, s0:s0 + 512], MUL)
                nc.vector.tensor_tensor(dst, t0[:], t1[:], ADD)

            at_sb = [[ap.tile([128, 512], bf16, tag=f"at{p}{h}", name=f"at_sb{p}{h}")
                      for h in range(HPC)] for p in range(2)]

            def norm_head(J, h, avp, sup):
                """softmax normalization of head h of block J: 1/sum broadcast
                via K=1 PE outer product, applied on DVE into bf16 at tile."""
                sums = smp.tile([1, 512], f32, tag="sums")
                nc.vector.tensor_copy(sums[:], sup[:])
                recip = smp.tile([1, 512], f32r, tag="recip")
                with nc.allow_low_precision(reason="f32r rounding of softmax denom, ~1e-4 rel"):
                    nc.vector.reciprocal(recip[:], sums[:])
                bcp = scps.tile([128, 512], f32, tag="sc", name="bcp")
                nc.tensor.matmul(bcp[:], onesrow_sb[:], recip[:], start=True, stop=True)
                bcs = bp.tile([128, 512], f32, tag="bcs")
                nc.scalar.activation(bcs[:], bcp[:], CPY)
                nc.vector.tensor_tensor(at_sb[J % 2][h][:], avp[:], bcs[:], MUL)

            def wo_block(J):
                """output projection for block J (deferred one block so its
                weights load late and its PE work overlaps attention's ACT)."""
                s0 = J * 512
                for i in range(4):
                    ost = fsb.tile([128, HID], bf16, tag="fo")
                    for e in range(4):
                        fp = scps.tile([128, 512], f32, tag="sc", name="fp")
                        for h in range(HPC):
                            nc.tensor.matmul(fp[:], at_sb[J % 2][h][:, i * 128:(i + 1) * 128],
                                             wo_sb[:, h, e * 512:(e + 1) * 512],
                                             start=(h == 0), stop=(h == HPC - 1))
                        if e % 2 == 0:
                            nc.vector.tensor_copy(ost[:, e * 512:(e + 1) * 512], fp[:])
                        else:
                            nc.scalar.activation(ost[:, e * 512:(e + 1) * 512], fp[:], CPY)
                    if J == NJ - 1 and i == 3:
                        # final row block: per-chunk DMAs so the tail drains sooner
                        for e in range(4):
                            nc.sync.dma_start(out[s0 + i * 128:s0 + (i + 1) * 128,
                                                  e * 512:(e + 1) * 512],
                                              ost[:, e * 512:(e + 1) * 512])
                    else:
                        nc.sync.dma_start(out[s0 + i * 128:s0 + (i + 1) * 128, :], ost[:])

            qpj = [0]

            def qpass(h, s0):
                # projection + rope for one q head, alternating pj banks
                t_q = pj.tile([128, 512], f32, tag=f"pj{qpj[0]}", name="pjq")
                qpj[0] ^= 1
                for ko in range(NKO):
                    st, sp = (ko == 0), (ko == NKO - 1)
                    nc.tensor.matmul(t_q[:], wq_sb[:, ko, h * D:(h + 1) * D],
                                     x_sb[:, ko, s0:s0 + 512], start=st, stop=sp)
                rope_math(q_sb[h][:, s0:s0 + 512], rope_cs(t_q), s0)

            pend = None   # (J, h, avp, sup) of the head awaiting normalization
            for J in range(NJ):
                s0 = J * 512
                # ---- projections for block J ----
                if J == 0:
                    # Two interleaved passes, ordered by DMA arrival: (q0,q1,k)
                    # needs only the first wq half + wk; (q2,q3) needs the
                    # second wq half which lands after the first x slice. The
                    # extra banks are borrowed from the still-idle "sc" tag.
                    qps = [pj.tile([128, 512], f32, tag="pj0", name="qps0"),
                           pj.tile([128, 512], f32, tag="pj1", name="qps1"),
                           scps.tile([128, 512], f32, tag="sc", name="qps2"),
                           scps.tile([128, 512], f32, tag="sc", name="qps3")]
                    kps0 = scps.tile([128, 512], f32, tag="sc", name="kps0")
                    for ko in range(NKO):
                        st, sp = (ko == 0), (ko == NKO - 1)
                        xt = x_sb[:, ko, s0:s0 + 512]
                        nc.tensor.matmul(qps[0][:], wq_sb[:, ko, 0:D], xt, start=st, stop=sp)
                        nc.tensor.matmul(qps[1][:], wq_sb[:, ko, D:2 * D], xt, start=st, stop=sp)
                        nc.tensor.matmul(kps0[:], wk_sb[:, ko, :], xt, start=st, stop=sp)
                    csq01 = [rope_cs(qps[0]), rope_cs(qps[1])]
                    csk = rope_cs(kps0)
                    vps0 = avps.tile([128, 512], f32, tag="av", name="vps0")
                    for sblk in range(4):
                        for ko in range(NKO):
                            st, sp = (ko == 0), (ko == NKO - 1)
                            nc.tensor.matmul(vps0[:, sblk * 128:(sblk + 1) * 128],
                                             x_sb[:, ko, s0 + sblk * 128:s0 + (sblk + 1) * 128],
                                             wv_sb[:, ko, :], start=st, stop=sp)
                    rope_math(q_sb[0][:, s0:s0 + 512], csq01[0], s0)
                    rope_math(k_sb[:, s0:s0 + 512], csk, s0)
                    for ko in range(NKO):
                        st, sp = (ko == 0), (ko == NKO - 1)
                        xt = x_sb[:, ko, s0:s0 + 512]
                        nc.tensor.matmul(qps[2][:], wq_sb[:, ko, 2 * D:3 * D], xt, start=st, stop=sp)
                        nc.tensor.matmul(qps[3][:], wq_sb[:, ko, 3 * D:4 * D], xt, start=st, stop=sp)
                    csq23 = [rope_cs(qps[2]), rope_cs(qps[3])]
                    rope_math(q_sb[1][:, s0:s0 + 512], csq01[1], s0)
                    rope_math(q_sb[2][:, s0:s0 + 512], csq23[0], s0)
                    rope_math(q_sb[3][:, s0:s0 + 512], csq23[1], s0)
                    nc.scalar.activation(v_sb[:, J * 4:(J + 1) * 4, :], vps0[:], CPY)
                else:
                    # k/v first, then one q head per pass. The q-passes for
                    # heads 1..3 are emitted INSIDE the attention head loop
                    # (below) so their PE work fills attention's ACT-bound
                    # stretches. Rope staging copies free the pj banks early.
                    kps = pj.tile([128, 512], f32, tag="pj0", name="kps")
                    for ko in range(NKO):
                        st, sp = (ko == 0), (ko == NKO - 1)
                        nc.tensor.matmul(kps[:], wk_sb[:, ko, :], x_sb[:, ko, s0:s0 + 512], start=st, stop=sp)
                    csk = rope_cs(kps)
                    rope_math(k_sb[:, s0:s0 + 512], csk, s0)
                    vps = pj.tile([128, 512], f32, tag="pj1", name="vps")
                    for sblk in range(4):
                        for ko in range(NKO):
                            st, sp = (ko == 0), (ko == NKO - 1)
                            nc.tensor.matmul(vps[:, sblk * 128:(sblk + 1) * 128],
                                             x_sb[:, ko, s0 + sblk * 128:s0 + (sblk + 1) * 128],
                                             wv_sb[:, ko, :], start=st, stop=sp)
                    nc.scalar.activation(v_sb[:, J * 4:(J + 1) * 4, :], vps[:], CPY)
                    qpass(0, s0)

                # ---- attention for block J ----
                nkb = 4 * J + 4
                for h in range(HPC):
                    avp = avps.tile([128, 512], f32, tag="av", name="avp")
                    sup = None
                    exq = []   # (jj, ex, off, n) awaiting their AV/sum matmuls

                    def flush_one():
                        nonlocal sup
                        jj0, ex0, off0, n0 = exq.pop(0)
                        st0, sp0 = (jj0 == 0), (jj0 == nkb - 1)
                        nc.tensor.matmul(avp[:, off0:512], v_sb[:, jj0, :], ex0[:, 0:n0], start=st0, stop=sp0)
                        if jj0 == 0:
                            sup = sups.tile([1, 512], f32, tag="su", name="sup")
                        nc.tensor.matmul(sup[:, off0:512], onescol_sb[:], ex0[:, 0:n0], start=st0, stop=sp0)

                    for jj in range(nkb):
                        p = jj - 4 * J
                        off = max(p, 0) * 128
                        n = 512 - off
                        scp = scps.tile([128, 512], f32, tag="sc", name="scp")
                        nc.tensor.matmul(scp[:, 0:n], k_sb[:, jj * 128:(jj + 1) * 128],
                                         q_sb[h][:, s0 + off:s0 + 512], start=True, stop=True)
                        if jj == 0 and pend is not None:
                            # normalize the previous head here: its PE op fills
                            # the exp-latency bubble of this head's first block
                            norm_head(*pend)
                            pend = None
                        if jj == min(1, nkb - 1) and J > 0 and h < HPC - 1:
                            # project the next head's q while ACT drains this
                            # head's exps
                            qpass(h + 1, s0)
                        ex = ep.tile([128, 512], bf16, tag="ex")
                        nc.scalar.activation(ex[:, 0:n], scp[:, 0:n], EXP,
                                             scale=kwsc_sb[:, h:h + 1])
                        if p >= 0:
                            # keep where col - row >= 0 (causal, off = p*128 aligns it)
                            nc.gpsimd.affine_select(
                                ex[:, 0:n], ex[:, 0:n], pattern=[[1, n]],
                                compare_op=mybir.AluOpType.is_ge, fill=0.0,
                                base=0, channel_multiplier=-1)
                        exq.append((jj, ex, off, n))
                        if jj >= 1:
                            # one-block skew: AV/sum of block jj-1 run while
                            # exp(jj) is still in flight on ACT
                            flush_one()
                    while exq:
                        flush_one()
                    pend = (J, h, avp, sup)

                # ---- deferred output projection of the previous block ----
                if J > 0:
                    wo_block(J - 1)
                norm_head(*pend)
                pend = None
            wo_block(NJ - 1)

    nc.compile()
    return nc


def _get_exec():
    """Build the Bass module once and wrap it in a cached jitted shard_map
    executable (mirrors concourse.bass2jax.run_bass_via_pjrt, minus donation so
    repeated calls can reuse device-resident buffers)."""
    if "exec" in _cache:
        return _cache["exec"]
    import jax
    import concourse.mybir as mybir
    from jax.experimental.shard_map import shard_map
    from jax.sharding import Mesh, PartitionSpec
    from concourse import bass2jax

    nc = _build()
    bass2jax.install_neuronx_cc_hook()

    partition_name = nc.partition_id_tensor.name if nc.partition_id_tensor else None
    in_names, out_names, out_avals = [], [], []
    for alloc in nc.m.functions[0].allocations:
        if not isinstance(alloc, mybir.__dict__["MemoryLocationSet"]):
            continue
        name = alloc.memorylocations[0].name
        if alloc.kind == "ExternalInput":
            if name != partition_name:
                in_names.append(name)
        elif alloc.kind == "ExternalOutput":
            out_names.append(name)
            out_avals.append(jax.core.ShapedArray(tuple(alloc.tensor_shape),
                                                  mybir.dt.np(alloc.dtype)))
    n_params = len(in_names)
    in_names = in_names + out_names  # zero-buffer operands, per bass2jax contract
    if partition_name is not None:
        in_names.append(partition_name)

    def _body(*args):
        operands = list(args)
        if partition_name is not None:
            operands.append(bass2jax.partition_id_tensor())
        outs = bass2jax._bass_exec_p.bind(
            *operands,
            out_avals=tuple(out_avals),
            in_names=tuple(in_names),
            out_names=tuple(out_names),
            lowering_input_output_aliases=(),
            sim_require_finite=True,
            sim_require_nnan=True,
            nc=nc,
        )
        return tuple(outs)

    devices = jax.devices()[:NCORES]
    mesh = Mesh(np.asarray(devices), ("core",))
    spec = PartitionSpec("core")
    sharded = jax.jit(
        shard_map(_body, mesh=mesh,
                  in_specs=(spec,) * (n_params + len(out_names)),
                  out_specs=(spec,) * len(out_names),
                  check_rep=False),
        keep_unused=True,
    )
    _cache["exec"] = {
        "sharded": sharded, "in_names": in_names, "out_names": out_names,
        "out_avals": out_avals, "n_params": n_params, "mesh": mesh, "spec": spec,
    }
    return _cache["exec"]


def _prep_in_maps(x, wq, wk, wv, wo, key_weights):
    x = np.asarray(x, dtype=np.float32)
    wq = np.asarray(wq, dtype=np.float32)
    wk = np.asarray(wk, dtype=np.float32)
    wv = np.asarray(wv, dtype=np.float32)
    wo = np.asarray(wo, dtype=np.float32)
    key_weights = np.asarray(key_weights, dtype=np.float32)

    wqT = np.ascontiguousarray(wq.T).astype(BF16)            # [HID, NH*D]
    wkT = np.ascontiguousarray(wk.T).astype(BF16)            # [HID, NKV*D]
    wvT = np.ascontiguousarray(wv.T).astype(BF16)
    woT = np.ascontiguousarray(wo.T).astype(BF16)            # [NH*D, HID]
    xTb = [np.ascontiguousarray(x[b].T).astype(BF16) for b in range(B)]

    in_maps = []
    for c in range(NCORES):
        b, g = c // NKV, c % NKV
        in_maps.append({
            "xT": xTb[b],
            "wqT": np.ascontiguousarray(wqT[:, g * HPC * D:(g + 1) * HPC * D]),
            "wkT": np.ascontiguousarray(wkT[:, g * D:(g + 1) * D]),
            "wvT": np.ascontiguousarray(wvT[:, g * D:(g + 1) * D]),
            "woT": np.ascontiguousarray(woT[g * HPC * D:(g + 1) * HPC * D, :]),
            "kw": np.ascontiguousarray(key_weights[g * HPC:(g + 1) * HPC]),
        })
    return in_maps


def _concat_args(ex, in_maps):
    concat_in = [
        np.concatenate([np.asarray(in_maps[c][name]) for c in range(NCORES)], axis=0)
        for name in ex["in_names"][:ex["n_params"]]
    ]
    zeros = [
        np.zeros((NCORES * av.shape[0], *av.shape[1:]), av.dtype)
        for av in ex["out_avals"]
    ]
    return concat_in + zeros


def kernel(x, wq, wk, wv, wo, key_weights):
    ex = _get_exec()
    in_maps = _prep_in_maps(x, wq, wk, wv, wo, key_weights)
    args = _concat_args(ex, in_maps)
    out_arrs = ex["sharded"](*args)
    parts = np.asarray(out_arrs[0]).astype(np.float32).reshape(B, NKV, S, HID)
    return parts.sum(axis=1)
